# revision 11
# baseline (speedup 1.0000x reference)
"""LSTM sequence classifier on 8 Trainium2 NeuronCores.

Data-parallel over batch: each core gets ~1/8 of the 4096 sequences.
Per core: dma_gather (transpose mode, 4 SWDGE queues) pulls token
embeddings from the bf16 table in HBM into feature-major SBUF layout;
a fully unrolled 22-step LSTM runs as bf16 matmuls (fp32 PSUM
accumulate). Biases ride in the matmul via a constant-1 embedding
column, so each gate drains from a 3-bank PSUM tile with a single
ACT instruction (sigmoid/tanh). Cell math runs on DVE in bf16 (2x
mode). Batches are sorted by length (descending) and dealt so all
cores share an identical length multiset; per-step work shrinks to
the still-active prefix.
"""
import sys

sys.path.insert(0, "/opt/trn_rl_repo")

import numpy as np
import ml_dtypes

import concourse.bass as bass
import concourse.tile as tile
from concourse import bacc, mybir
from concourse.bass_utils import run_bass_kernel_spmd

V, E, H, T, B = 30000, 300, 300, 22, 4096
NCORES = 8
EP = 384          # padded embedding row (elements); 768 B in bf16
GP = 384          # padded rows per gate (3 K-tiles of 128)
MW = 4 * GP       # 1536 padded gate rows total
NMT = MW // 128   # 12 M-tiles
KT = 3            # K-tiles per operand (301 -> 128,128,45 incl bias row)
F32 = mybir.dt.float32
BF16 = mybir.dt.bfloat16
I16 = mybir.dt.int16
AF = mybir.ActivationFunctionType

_patched = False


def _patch_tile_drain():
    """walrus CTRL (Drain) supports fewer sem waits than Tile attaches at
    the kernel tail; spread them across single-wait SP NOPs instead."""
    global _patched
    if _patched:
        return
    _patched = True
    import concourse.tile as tile_mod
    from concourse.vector_clock import ScopedClock

    def _drain_and_barrier(self, tick_clock, wait_clock):
        nc = self.nc
        probe = nc.sync.nop(nofuse=True)
        wait_clock.add_sem_waits(
            probe.ins, ScopedClock({None: tick_clock.global_clock}))
        si = probe.ins.sync_info
        waits = list(si.on_wait) if si is not None else []
        upds = list(si.on_update) if si is not None else []
        probe.ins.sync_info = mybir.SyncInfo(on_wait=waits[:1], on_update=upds)
        for w in waits[1:]:
            n2 = nc.sync.nop(nofuse=True)
            n2.ins.sync_info = mybir.SyncInfo(on_wait=[w], on_update=[])
        nc.sync.drain()
        nc.all_engine_barrier()
        popped = nc._tile_sem_poison_stack.pop()
        assert popped is self._sem_poison
        nc.clear_and_free_semaphores(list(self.sems.allocated().values()))
        nc.all_engine_barrier()

    tile_mod.TileContext._drain_and_barrier = _drain_and_barrier


def _schedule(cap_len):
    """Deal batches to cores so every core has the same length multiset.

    Returns orders ([NCORES][Q] of global index or -1 for dummy) and the
    per-step active counts n_t (identical across cores).
    """
    q = np.zeros(T + 1, np.int64)  # q[l] = per-core count of length l
    orders = [[] for _ in range(NCORES)]
    for l in range(T, 0, -1):
        idxs = np.nonzero(cap_len == l)[0]
        k = len(idxs)
        ql = -(-k // NCORES)  # ceil
        q[l] = ql
        for c in range(NCORES):
            part = idxs[c::NCORES]
            orders[c].extend(int(x) for x in part)
            orders[c].extend([-1] * (ql - len(part)))
    n_t = [int(q[t + 1:].sum()) for t in range(T)]  # active at step t
    return orders, n_t


DEBUG_DUMP = False
DRAIN_SPLIT = False


def _build_program(n_t, Q, NTOKP, chunks, offs):
    nc = bacc.Bacc("TRN2", target_bir_lowering=False, debug=False)
    emb_d = nc.dram_tensor("emb", [V, EP], BF16, kind="ExternalInput")
    idx_d = nc.dram_tensor("idx", [128, NTOKP // 16], I16, kind="ExternalInput")
    wx_d = nc.dram_tensor("wx", [KT, 128, MW], BF16, kind="ExternalInput")
    wh_d = nc.dram_tensor("wh", [KT, 128, MW], BF16, kind="ExternalInput")
    vt_d = nc.dram_tensor("vt", [KT, 128, 2], BF16, kind="ExternalInput")
    bc_d = nc.dram_tensor("bc", [2, 1], F32, kind="ExternalInput")
    out_d = nc.dram_tensor("out", [2, Q], F32, kind="ExternalOutput")
    if DEBUG_DUMP:
        QR0 = -(-Q // 8) * 8
        dbg_g = nc.dram_tensor("dbg_g", [4, 128, KT, QR0], F32,
                               kind="ExternalOutput")
        dbg_c = nc.dram_tensor("dbg_c", [2, 128, KT, QR0], F32,
                               kind="ExternalOutput")

    QR = -(-Q // 8) * 8
    SP = Q > 512           # spill columns beyond 512
    SPW = max(8, QR - 512) if SP else 0
    # gate order in the M layout: i, f, g, o (matches reference split)
    # issue order per step: i, g, f, o
    GFUNC = {0: AF.Sigmoid, 1: AF.Sigmoid, 2: AF.Tanh, 3: AF.Sigmoid}
    ISSUE = [0, 2, 1, 3]

    with tile.TileContext(nc) as tc:
        with (
            tc.tile_pool(name="const", bufs=1) as cpool,
            tc.tile_pool(name="xt", bufs=1) as xpool,
            tc.tile_pool(name="state", bufs=1) as spool,
            tc.tile_pool(name="ps", bufs=2, space="PSUM") as pspool,
            tc.tile_pool(name="spill", bufs=1, space="PSUM") as sppool,
            tc.tile_pool(name="psh", bufs=1, space="PSUM") as hpool,
        ):
            # ACT table warm-up: load the sigmoid/tanh table set ASAP so it
            # doesn't serialize behind the first gather.
            warm = spool.tile([2, 8], F32, tag="warm")
            nc.vector.memset(warm[:], 0.0)
            nc.scalar.activation(warm[:], warm[:], AF.Sigmoid)

            wx_sb = cpool.tile([128, KT, MW], BF16, tag="wx")
            wh_sb = cpool.tile([128, KT, MW], BF16, tag="wh")
            for k in range(KT):
                nc.sync.dma_start(out=wx_sb[:, k, :], in_=wx_d[k])
                nc.sync.dma_start(out=wh_sb[:, k, :], in_=wh_d[k])
            vt_sb = cpool.tile([128, KT, 2], BF16, tag="vt")
            for k in range(KT):
                nc.sync.dma_start(out=vt_sb[:, k, :], in_=vt_d[k])
            bc_sb = cpool.tile([2, 1], F32, tag="bc")
            nc.sync.dma_start(out=bc_sb[:], in_=bc_d[:])
            idx_sb = cpool.tile([128, NTOKP // 16], I16, tag="idx")
            nc.sync.dma_start(out=idx_sb[:], in_=idx_d[:])

            # gather chunks (feature-major bf16: xt[q, c, i] = emb[tok_i, 128c+q])
            # spread across the 4 SWDGE queues so they land in parallel
            xts = []
            for ci, (s0, s1) in enumerate(chunks):
                xt = xpool.tile([128, KT, s1 - s0], BF16, tag=f"xt{ci}")
                nc.gpsimd.dma_gather(
                    out_ap=xt[:], in_ap=emb_d[:],
                    idxs_ap=idx_sb[:, s0 // 16:s1 // 16],
                    num_idxs=s1 - s0, num_idxs_reg=s1 - s0,
                    elem_size=EP, transpose=True, single_packet=False)
                xts.append(xt)

            hT = spool.tile([128, KT, QR], BF16, tag="hT")
            cT = spool.tile([128, KT, QR], BF16, tag="cT")
            tanh_c = spool.tile([128, KT, QR], BF16, tag="tanh_c")
            tmp = spool.tile([128, KT, QR], BF16, tag="tmp")
            lastT = spool.tile([128, KT, QR], BF16, tag="lastT")
            gbufs = []
            for nm in ["ib", "fb", "gb", "ob"]:
                gt = spool.tile([128, KT, QR], BF16, tag=nm, name=nm)
                gbufs.append(gt)

            def x_segments(off, lo, hi):
                """split cols [lo,hi) at gather-chunk crossings and col 512
                (main PSUM tile vs spill tile boundary)"""
                segs = []
                col = lo
                while col < hi:
                    p = off + col
                    ci = next(i for i, (c0, c1) in enumerate(chunks)
                              if c0 <= p < c1)
                    end = min(hi, chunks[ci][1] - off)
                    if col < 512:
                        end = min(end, 512)
                    segs.append((col, end, ci, p - chunks[ci][0]))
                    col = end
                return segs

            def gate_mms(t, g, off, n, ps, ps_sp):
                """Issue all matmuls for gate g of step t.

                start=True clears has_written for the WHOLE bank, so a
                bank's (x+h) group for one column segment must fully stop
                before the next segment's group starts in that bank. Within
                one segment the x-phase is front-loaded across the 3 banks
                (independent), then the h-phase closes each bank's group.
                """
                has_h = t > 0
                for (lo, hi, ci, a) in x_segments(off, 0, n):
                    w = hi - lo
                    if lo >= 512:
                        pst, jlo = ps_sp, 512
                    else:
                        pst, jlo = ps, 0
                    for j in range(3):
                        m = g * 3 + j
                        for k in range(KT):
                            nc.tensor.matmul(
                                pst[:, j, lo - jlo:hi - jlo],
                                wx_sb[:, k, m * 128:(m + 1) * 128],
                                xts[ci][:, k, a:a + w],
                                start=(k == 0),
                                stop=(not has_h and k == KT - 1))
                    if has_h:
                        for j in range(3):
                            m = g * 3 + j
                            for k in range(KT):
                                nc.tensor.matmul(
                                    pst[:, j, lo - jlo:hi - jlo],
                                    wh_sb[:, k, m * 128:(m + 1) * 128],
                                    hT[:, k, lo:hi],
                                    start=False, stop=(k == KT - 1))

            def gate_drain(g, n, ps, ps_sp):
                nm = min(n, 512)
                if DRAIN_SPLIT:
                    for j in range(3):
                        nc.scalar.activation(gbufs[g][:, j, 0:nm],
                                             ps[:, j, 0:nm], GFUNC[g])
                else:
                    nc.scalar.activation(gbufs[g][:, :, 0:nm], ps[:, :, 0:nm],
                                         GFUNC[g])
                if n > 512:
                    nc.scalar.activation(gbufs[g][:, :, 512:n],
                                         ps_sp[:, :, 0:n - 512], GFUNC[g])

            for t in range(T):
                n = n_t[t]
                if n == 0:
                    continue
                off = offs[t]
                ib, fb, gb, ob = gbufs
                pss = {}
                spills = {}
                for gi, g in enumerate(ISSUE):
                    ps = pspool.tile([128, 3, 512], F32, tag="ps", name="ps")
                    ps_sp = None
                    if n > 512:
                        ps_sp = sppool.tile([128, 3, SPW], F32, tag="sp",
                                            name="ps_sp")
                    pss[g] = ps
                    spills[g] = ps_sp
                    gate_mms(t, g, off, n, ps, ps_sp)
                    gate_drain(g, n, ps, ps_sp)
                    # interleave DVE cell math as inputs become ready
                    if gi == 1:  # i and g drained
                        if t == 0:
                            nc.vector.tensor_mul(cT[:, :, :n], ib[:, :, :n],
                                                 gb[:, :, :n])
                        else:
                            nc.vector.tensor_mul(tmp[:, :, :n], ib[:, :, :n],
                                                 gb[:, :, :n])
                    if gi == 2:  # f drained
                        if t > 0:
                            nc.vector.tensor_mul(cT[:, :, :n], fb[:, :, :n],
                                                 cT[:, :, :n])
                            nc.vector.tensor_add(cT[:, :, :n], cT[:, :, :n],
                                                 tmp[:, :, :n])
                        nc.scalar.activation(tanh_c[:, :, :n], cT[:, :, :n],
                                             AF.Tanh)
                # o drained: produce h and capture finished sequences
                cap_lo = n_t[t + 1] if t < T - 1 else 0
                if t < T - 1 and cap_lo > 0:
                    nc.vector.tensor_mul(hT[:, :, :cap_lo], ob[:, :, :cap_lo],
                                         tanh_c[:, :, :cap_lo])
                if cap_lo < n:
                    nc.vector.tensor_mul(lastT[:, :, cap_lo:n],
                                         ob[:, :, cap_lo:n],
                                         tanh_c[:, :, cap_lo:n])
                if DEBUG_DUMP and t == 0:
                    dbg_sb = spool.tile([128, KT, QR], F32, tag="dbg_sb")
                    for gi in range(4):
                        nc.vector.tensor_copy(dbg_sb[:], gbufs[gi][:])
                        nc.sync.dma_start(out=dbg_g[gi], in_=dbg_sb[:])
                    nc.vector.tensor_copy(dbg_sb[:], cT[:])
                    nc.sync.dma_start(out=dbg_c[0], in_=dbg_sb[:])
                    nc.vector.tensor_copy(dbg_sb[:], hT[:])
                    nc.sync.dma_start(out=dbg_c[1], in_=dbg_sb[:])

            # head: logits^T = W @ last^T + b_cls (W precomputed on host)
            out_sb = spool.tile([2, QR], F32, tag="out_sb")
            col = 0
            while col < Q:
                w = min(512, Q - col)
                ph = hpool.tile([2, 512], F32, tag="ph")
                for k in range(KT):
                    nc.tensor.matmul(ph[:, :w], vt_sb[:, k, :],
                                     lastT[:, k, col:col + w],
                                     start=(k == 0), stop=(k == KT - 1))
                nc.scalar.activation(out_sb[:, col:col + w], ph[:, :w],
                                     AF.Identity, bias=bc_sb[:, 0:1],
                                     scale=1.0)
                col += w
            nc.sync.dma_start(out=out_d[:], in_=out_sb[:, :Q])

    nc.compile()
    return nc


def _prep_and_run(inputs, trace=False):
    _patch_tile_drain()
    cap = np.asarray(inputs["cap"]).astype(np.int64)
    cap_len = np.asarray(inputs["cap_len"]).astype(np.int64)
    embed = np.asarray(inputs["embed"], np.float32)
    W_ih = np.asarray(inputs["W_ih"], np.float32)
    W_hh = np.asarray(inputs["W_hh"], np.float32)
    b_ih = np.asarray(inputs["b_ih"], np.float32)
    b_hh = np.asarray(inputs["b_hh"], np.float32)
    v_wn = np.asarray(inputs["v_wn"], np.float32)
    g_wn = np.asarray(inputs["g_wn"], np.float32)
    b_cls = np.asarray(inputs["b_cls"], np.float32)

    orders, n_t = _schedule(cap_len)
    Q = n_t[0]
    offs = np.concatenate([[0], np.cumsum(n_t)]).astype(np.int64)
    NTOK = int(offs[-1])
    NTOKP = -(-NTOK // 128) * 128

    # per-core token streams, packed for dma_gather (idx i -> [i%16, i//16])
    idx_maps = []
    for c in range(NCORES):
        order = np.asarray(orders[c], np.int64)
        toks = np.zeros(NTOKP, np.int16)
        for t in range(T):
            n = n_t[t]
            sel = order[:n]
            tk = np.where(sel >= 0, cap[np.clip(sel, 0, None), t], 0)
            toks[offs[t]:offs[t] + n] = tk.astype(np.int16)
        packed = np.tile(toks.reshape(NTOKP // 16, 16).T, (8, 1)).copy()
        idx_maps.append(packed)

    # graded chunks: small first chunks (4 queues land them in parallel)
    sizes = [256, 256, 512, 512, 1024, 1024]
    chunks = []
    s = 0
    while s < NTOKP:
        cl = sizes.pop(0) if sizes else 1536
        chunks.append((s, min(s + cl, NTOKP)))
        s += cl

    # embedding table: bf16, padded to EP with a constant-1 bias column at
    # index 300 (so W rows at k-row 300 add the LSTM bias inside the matmul)
    emb_pad = np.zeros((V, EP), ml_dtypes.bfloat16)
    emb_pad[:, :E] = embed.astype(ml_dtypes.bfloat16)
    emb_pad[:, E] = np.float32(1.0)

    def pack_w(Wmat, kdim, bias=None):
        Wp = np.zeros((MW, EP), np.float32)
        for g in range(4):
            Wp[GP * g:GP * g + H, :kdim] = Wmat[H * g:H * g + H, :]
            if bias is not None:
                Wp[GP * g:GP * g + H, E] = bias[H * g:H * g + H]
        return np.ascontiguousarray(
            Wp.T.reshape(KT, 128, MW)).astype(ml_dtypes.bfloat16)

    wx_np = pack_w(W_ih, E, bias=(b_ih + b_hh))
    wh_np = pack_w(W_hh, H)

    # weight-normed head, computed on host: W = g * v / ||v||
    Wv = (g_wn[:, None] * v_wn / np.linalg.norm(v_wn, axis=1, keepdims=True))
    v_pad = np.zeros((2, EP), np.float32)
    v_pad[:, :H] = Wv
    vt_np = np.ascontiguousarray(
        v_pad.T.reshape(KT, 128, 2)).astype(ml_dtypes.bfloat16)
    bc_np = np.ascontiguousarray(b_cls.reshape(2, 1)).astype(np.float32)

    nc = _build_program(n_t, Q, NTOKP, chunks, offs)

    in_maps = []
    for c in range(NCORES):
        in_maps.append({
            "emb": emb_pad, "idx": idx_maps[c], "wx": wx_np, "wh": wh_np,
            "vt": vt_np, "bc": bc_np,
        })
    res = run_bass_kernel_spmd(nc, in_maps, list(range(NCORES)), trace=trace)

    out = np.zeros((B, 2), np.float32)
    for c in range(NCORES):
        logitsT = res.results[c]["out"]  # [2, Q]
        order = orders[c]
        for pos, gi in enumerate(order):
            if gi >= 0:
                out[gi] = logitsT[:, pos]
    return out, res


def kernel(**inputs):
    out, _ = _prep_and_run(inputs, trace=False)
    return out


# revision 16
# speedup vs baseline: 1.0547x; 1.0547x over previous
"""LSTM sequence classifier on 8 Trainium2 NeuronCores.

Data-parallel over batch: each core gets ~1/8 of the 4096 sequences.
Per core: dma_gather (transpose mode, 4 SWDGE queues) pulls token
embeddings from the bf16 table in HBM into feature-major SBUF layout;
a fully unrolled 22-step LSTM runs as bf16 matmuls (fp32 PSUM
accumulate). Biases ride in the matmul via a constant-1 embedding
column, so each gate drains from a 3-bank PSUM tile with a single
ACT instruction (sigmoid/tanh). Cell math runs on DVE in bf16 (2x
mode). Batches are sorted by length (descending) and dealt so all
cores share an identical length multiset; per-step work shrinks to
the still-active prefix.
"""
import sys

sys.path.insert(0, "/opt/trn_rl_repo")

import numpy as np
import ml_dtypes

import concourse.bass as bass
import concourse.tile as tile
from concourse import bacc, mybir
from concourse.bass_utils import run_bass_kernel_spmd

V, E, H, T, B = 30000, 300, 300, 22, 4096
NCORES = 8
EP = 384          # padded embedding row (elements); 768 B in bf16
GP = 384          # padded rows per gate (3 K-tiles of 128)
MW = 4 * GP       # 1536 padded gate rows total
NMT = MW // 128   # 12 M-tiles
KT = 3            # K-tiles per operand (301 -> 128,128,45 incl bias row)
F32 = mybir.dt.float32
BF16 = mybir.dt.bfloat16
I16 = mybir.dt.int16
AF = mybir.ActivationFunctionType

_patched = False


def _patch_tile_drain():
    """walrus CTRL (Drain) supports fewer sem waits than Tile attaches at
    the kernel tail; spread them across single-wait SP NOPs instead."""
    global _patched
    if _patched:
        return
    _patched = True
    import concourse.tile as tile_mod
    from concourse.vector_clock import ScopedClock

    def _drain_and_barrier(self, tick_clock, wait_clock):
        nc = self.nc
        probe = nc.sync.nop(nofuse=True)
        wait_clock.add_sem_waits(
            probe.ins, ScopedClock({None: tick_clock.global_clock}))
        si = probe.ins.sync_info
        waits = list(si.on_wait) if si is not None else []
        upds = list(si.on_update) if si is not None else []
        probe.ins.sync_info = mybir.SyncInfo(on_wait=waits[:1], on_update=upds)
        for w in waits[1:]:
            n2 = nc.sync.nop(nofuse=True)
            n2.ins.sync_info = mybir.SyncInfo(on_wait=[w], on_update=[])
        nc.sync.drain()
        nc.all_engine_barrier()
        popped = nc._tile_sem_poison_stack.pop()
        assert popped is self._sem_poison
        nc.clear_and_free_semaphores(list(self.sems.allocated().values()))
        nc.all_engine_barrier()

    tile_mod.TileContext._drain_and_barrier = _drain_and_barrier


def _schedule(cap_len):
    """Deal batches to cores so every core has the same length multiset.

    Returns orders ([NCORES][Q] of global index or -1 for dummy) and the
    per-step active counts n_t (identical across cores).
    """
    q = np.zeros(T + 1, np.int64)  # q[l] = per-core count of length l
    orders = [[] for _ in range(NCORES)]
    for l in range(T, 0, -1):
        idxs = np.nonzero(cap_len == l)[0]
        k = len(idxs)
        ql = -(-k // NCORES)  # ceil
        q[l] = ql
        for c in range(NCORES):
            part = idxs[c::NCORES]
            orders[c].extend(int(x) for x in part)
            orders[c].extend([-1] * (ql - len(part)))
    n_t = [int(q[t + 1:].sum()) for t in range(T)]  # active at step t
    return orders, n_t


DEBUG_DUMP = False
DRAIN_SPLIT = False


def _build_program(n_t, Q, NTOKP, chunks, offs):
    nc = bacc.Bacc("TRN2", target_bir_lowering=False, debug=False)
    emb_d = nc.dram_tensor("emb", [V, EP], BF16, kind="ExternalInput")
    idx_d = nc.dram_tensor("idx", [128, NTOKP // 16], I16, kind="ExternalInput")
    wx_d = nc.dram_tensor("wx", [KT, 128, MW], BF16, kind="ExternalInput")
    wh_d = nc.dram_tensor("wh", [KT, 128, MW], BF16, kind="ExternalInput")
    vt_d = nc.dram_tensor("vt", [KT, 128, 2], BF16, kind="ExternalInput")
    bc_d = nc.dram_tensor("bc", [2, 1], F32, kind="ExternalInput")
    out_d = nc.dram_tensor("out", [2, Q], F32, kind="ExternalOutput")
    if DEBUG_DUMP:
        QR0 = -(-Q // 8) * 8
        dbg_g = nc.dram_tensor("dbg_g", [4, 128, KT, QR0], F32,
                               kind="ExternalOutput")
        dbg_c = nc.dram_tensor("dbg_c", [2, 128, KT, QR0], F32,
                               kind="ExternalOutput")

    QR = -(-Q // 8) * 8
    SP = Q > 512           # spill columns beyond 512
    SPW = max(8, QR - 512) if SP else 0
    # gate order in the M layout: i, f, g, o (matches reference split)
    # issue order per step: i, g, f, o
    GFUNC = {0: AF.Sigmoid, 1: AF.Sigmoid, 2: AF.Tanh, 3: AF.Sigmoid}
    ISSUE = [0, 2, 1, 3]

    with tile.TileContext(nc) as tc:
        with (
            tc.tile_pool(name="const", bufs=1) as cpool,
            tc.tile_pool(name="xt", bufs=1) as xpool,
            tc.tile_pool(name="state", bufs=1) as spool,
            tc.tile_pool(name="ps", bufs=2, space="PSUM") as pspool,
            tc.tile_pool(name="spill", bufs=1, space="PSUM") as sppool,
            tc.tile_pool(name="psh", bufs=1, space="PSUM") as hpool,
        ):
            # ACT table warm-up: load the sigmoid/tanh table set ASAP so it
            # doesn't serialize behind the first gather.
            warm = spool.tile([2, 8], F32, tag="warm")
            nc.vector.memset(warm[:], 0.0)
            nc.scalar.activation(warm[:], warm[:], AF.Sigmoid)

            wx_sb = cpool.tile([128, KT, MW], BF16, tag="wx")
            wh_sb = cpool.tile([128, KT, MW], BF16, tag="wh")
            for k in range(KT):
                nc.sync.dma_start(out=wx_sb[:, k, :], in_=wx_d[k])
                nc.sync.dma_start(out=wh_sb[:, k, :], in_=wh_d[k])
            vt_sb = cpool.tile([128, KT, 2], BF16, tag="vt")
            for k in range(KT):
                nc.sync.dma_start(out=vt_sb[:, k, :], in_=vt_d[k])
            bc_sb = cpool.tile([2, 1], F32, tag="bc")
            nc.sync.dma_start(out=bc_sb[:], in_=bc_d[:])
            idx_sb = cpool.tile([128, NTOKP // 16], I16, tag="idx")
            nc.sync.dma_start(out=idx_sb[:], in_=idx_d[:])

            # gather chunks (feature-major bf16: xt[q, c, i] = emb[tok_i, 128c+q])
            # spread across the 4 SWDGE queues so they land in parallel
            xts = []
            for ci, (s0, s1) in enumerate(chunks):
                xt = xpool.tile([128, KT, s1 - s0], BF16, tag=f"xt{ci}")
                nc.gpsimd.dma_gather(
                    out_ap=xt[:], in_ap=emb_d[:],
                    idxs_ap=idx_sb[:, s0 // 16:s1 // 16],
                    num_idxs=s1 - s0, num_idxs_reg=s1 - s0,
                    elem_size=EP, transpose=True, single_packet=False)
                xts.append(xt)

            hT = spool.tile([128, KT, QR], BF16, tag="hT")
            cT = spool.tile([128, KT, QR], BF16, tag="cT")
            tanh_c = spool.tile([128, KT, QR], BF16, tag="tanh_c")
            tmp = spool.tile([128, KT, QR], BF16, tag="tmp")
            lastT = spool.tile([128, KT, QR], BF16, tag="lastT")
            gbufs = []
            for nm in ["ib", "fb", "gb", "ob"]:
                gt = spool.tile([128, KT, QR], BF16, tag=nm, name=nm)
                gbufs.append(gt)

            def x_segments(off, lo, hi):
                """split cols [lo,hi) at gather-chunk crossings and col 512
                (main PSUM tile vs spill tile boundary). Chunks overlap in
                token space; prefer the chunk that covers the most columns."""
                segs = []
                col = lo
                while col < hi:
                    p = off + col
                    ci = max((i for i, (c0, c1) in enumerate(chunks)
                              if c0 <= p < c1),
                             key=lambda i: chunks[i][1])
                    end = min(hi, chunks[ci][1] - off)
                    if col < 512:
                        end = min(end, 512)
                    segs.append((col, end, ci, p - chunks[ci][0]))
                    col = end
                return segs

            def gate_mms(t, g, off, n, ps, ps_sp):
                """Issue all matmuls for gate g of step t.

                start=True clears has_written for the WHOLE bank, so a
                bank's (x+h) group for one column segment must fully stop
                before the next segment's group starts in that bank. Within
                one segment the x-phase is front-loaded across the 3 banks
                (independent), then the h-phase closes each bank's group.
                """
                has_h = t > 0
                for (lo, hi, ci, a) in x_segments(off, 0, n):
                    w = hi - lo
                    if lo >= 512:
                        pst, jlo = ps_sp, 512
                    else:
                        pst, jlo = ps, 0
                    for j in range(3):
                        m = g * 3 + j
                        for k in range(KT):
                            nc.tensor.matmul(
                                pst[:, j, lo - jlo:hi - jlo],
                                wx_sb[:, k, m * 128:(m + 1) * 128],
                                xts[ci][:, k, a:a + w],
                                start=(k == 0),
                                stop=(not has_h and k == KT - 1))
                    if has_h:
                        for j in range(3):
                            m = g * 3 + j
                            for k in range(KT):
                                nc.tensor.matmul(
                                    pst[:, j, lo - jlo:hi - jlo],
                                    wh_sb[:, k, m * 128:(m + 1) * 128],
                                    hT[:, k, lo:hi],
                                    start=False, stop=(k == KT - 1))

            def gate_drain(g, n, ps, ps_sp, split=False):
                nm = min(n, 512)
                if split or DRAIN_SPLIT:
                    # per-bank drains pipeline with per-bank h-writes
                    for j in range(3):
                        nc.scalar.activation(gbufs[g][:, j, 0:nm],
                                             ps[:, j, 0:nm], GFUNC[g])
                else:
                    nc.scalar.activation(gbufs[g][:, :, 0:nm], ps[:, :, 0:nm],
                                         GFUNC[g])
                if n > 512:
                    nc.scalar.activation(gbufs[g][:, :, 512:n],
                                         ps_sp[:, :, 0:n - 512], GFUNC[g])

            for t in range(T):
                n = n_t[t]
                if n == 0:
                    continue
                off = offs[t]
                ib, fb, gb, ob = gbufs
                pss = {}
                spills = {}
                for gi, g in enumerate(ISSUE):
                    ps = pspool.tile([128, 3, 512], F32, tag="ps", name="ps")
                    ps_sp = None
                    if n > 512:
                        ps_sp = sppool.tile([128, 3, SPW], F32, tag="sp",
                                            name="ps_sp")
                    pss[g] = ps
                    spills[g] = ps_sp
                    gate_mms(t, g, off, n, ps, ps_sp)
                    gate_drain(g, n, ps, ps_sp, split=(g == 3 and t < T - 1))
                    # interleave DVE cell math as inputs become ready
                    if gi == 1:  # i and g drained
                        if t == 0:
                            nc.vector.tensor_mul(cT[:, :, :n], ib[:, :, :n],
                                                 gb[:, :, :n])
                        else:
                            nc.vector.tensor_mul(tmp[:, :, :n], ib[:, :, :n],
                                                 gb[:, :, :n])
                    if gi == 2:  # f drained
                        if t > 0:
                            nc.vector.tensor_mul(cT[:, :, :n], fb[:, :, :n],
                                                 cT[:, :, :n])
                            nc.vector.tensor_add(cT[:, :, :n], cT[:, :, :n],
                                                 tmp[:, :, :n])
                        nc.scalar.activation(tanh_c[:, :, :n], cT[:, :, :n],
                                             AF.Tanh)
                # o drained: produce h and capture finished sequences
                cap_lo = n_t[t + 1] if t < T - 1 else 0
                if t < T - 1 and cap_lo > 0:
                    for j in range(3):
                        nc.vector.tensor_mul(hT[:, j, :cap_lo],
                                             ob[:, j, :cap_lo],
                                             tanh_c[:, j, :cap_lo])
                if cap_lo < n:
                    nc.vector.tensor_mul(lastT[:, :, cap_lo:n],
                                         ob[:, :, cap_lo:n],
                                         tanh_c[:, :, cap_lo:n])
                if DEBUG_DUMP and t == 0:
                    dbg_sb = spool.tile([128, KT, QR], F32, tag="dbg_sb")
                    for gi in range(4):
                        nc.vector.tensor_copy(dbg_sb[:], gbufs[gi][:])
                        nc.sync.dma_start(out=dbg_g[gi], in_=dbg_sb[:])
                    nc.vector.tensor_copy(dbg_sb[:], cT[:])
                    nc.sync.dma_start(out=dbg_c[0], in_=dbg_sb[:])
                    nc.vector.tensor_copy(dbg_sb[:], hT[:])
                    nc.sync.dma_start(out=dbg_c[1], in_=dbg_sb[:])

            # head: logits^T = W @ last^T + b_cls (W precomputed on host)
            out_sb = spool.tile([2, QR], F32, tag="out_sb")
            col = 0
            while col < Q:
                w = min(512, Q - col)
                ph = hpool.tile([2, 512], F32, tag="ph")
                for k in range(KT):
                    nc.tensor.matmul(ph[:, :w], vt_sb[:, k, :],
                                     lastT[:, k, col:col + w],
                                     start=(k == 0), stop=(k == KT - 1))
                nc.scalar.activation(out_sb[:, col:col + w], ph[:, :w],
                                     AF.Identity, bias=bc_sb[:, 0:1],
                                     scale=1.0)
                col += w
            nc.sync.dma_start(out=out_d[:], in_=out_sb[:, :Q])

    nc.compile()
    return nc


def _prep_and_run(inputs, trace=False):
    _patch_tile_drain()
    cap = np.asarray(inputs["cap"]).astype(np.int64)
    cap_len = np.asarray(inputs["cap_len"]).astype(np.int64)
    embed = np.asarray(inputs["embed"], np.float32)
    W_ih = np.asarray(inputs["W_ih"], np.float32)
    W_hh = np.asarray(inputs["W_hh"], np.float32)
    b_ih = np.asarray(inputs["b_ih"], np.float32)
    b_hh = np.asarray(inputs["b_hh"], np.float32)
    v_wn = np.asarray(inputs["v_wn"], np.float32)
    g_wn = np.asarray(inputs["g_wn"], np.float32)
    b_cls = np.asarray(inputs["b_cls"], np.float32)

    orders, n_t = _schedule(cap_len)
    Q = n_t[0]
    offs = np.concatenate([[0], np.cumsum(n_t)]).astype(np.int64)
    NTOK = int(offs[-1])
    NTOKP = -(-NTOK // 128) * 128

    # per-core token streams, packed for dma_gather (idx i -> [i%16, i//16])
    idx_maps = []
    for c in range(NCORES):
        order = np.asarray(orders[c], np.int64)
        toks = np.zeros(NTOKP, np.int16)
        for t in range(T):
            n = n_t[t]
            sel = order[:n]
            tk = np.where(sel >= 0, cap[np.clip(sel, 0, None), t], 0)
            toks[offs[t]:offs[t] + n] = tk.astype(np.int16)
        packed = np.tile(toks.reshape(NTOKP // 16, 16).T, (8, 1)).copy()
        idx_maps.append(packed)

    # step-aligned overlapping gather chunks: each chunk is a 128-aligned
    # superset of a group of steps' token ranges, so steps t>=1 read their
    # x in a single segment (no extra matmul splits at chunk crossings).
    def fl128(v):
        return (v // 128) * 128

    def cl128(v):
        return -(-v // 128) * 128

    chunks = []
    e0 = min(cl128(int(offs[1])), NTOKP)
    if e0 > 256:
        chunks += [(0, 256), (256, e0)]
    else:
        chunks += [(0, max(e0, 128))]
    for grp in ([1], [2, 3], [4, 5, 6], [7, 8, 9, 10], list(range(11, T))):
        ta, tb = grp[0], grp[-1]
        lo, hi = int(offs[ta]), int(offs[tb + 1])
        if hi <= lo:
            continue
        chunks.append((fl128(lo), min(cl128(hi), NTOKP)))

    # embedding table: bf16, padded to EP with a constant-1 bias column at
    # index 300 (so W rows at k-row 300 add the LSTM bias inside the matmul)
    emb_pad = np.zeros((V, EP), ml_dtypes.bfloat16)
    emb_pad[:, :E] = embed.astype(ml_dtypes.bfloat16)
    emb_pad[:, E] = np.float32(1.0)

    def pack_w(Wmat, kdim, bias=None):
        Wp = np.zeros((MW, EP), np.float32)
        for g in range(4):
            Wp[GP * g:GP * g + H, :kdim] = Wmat[H * g:H * g + H, :]
            if bias is not None:
                Wp[GP * g:GP * g + H, E] = bias[H * g:H * g + H]
        return np.ascontiguousarray(
            Wp.T.reshape(KT, 128, MW)).astype(ml_dtypes.bfloat16)

    wx_np = pack_w(W_ih, E, bias=(b_ih + b_hh))
    wh_np = pack_w(W_hh, H)

    # weight-normed head, computed on host: W = g * v / ||v||
    Wv = (g_wn[:, None] * v_wn / np.linalg.norm(v_wn, axis=1, keepdims=True))
    v_pad = np.zeros((2, EP), np.float32)
    v_pad[:, :H] = Wv
    vt_np = np.ascontiguousarray(
        v_pad.T.reshape(KT, 128, 2)).astype(ml_dtypes.bfloat16)
    bc_np = np.ascontiguousarray(b_cls.reshape(2, 1)).astype(np.float32)

    nc = _build_program(n_t, Q, NTOKP, chunks, offs)

    in_maps = []
    for c in range(NCORES):
        in_maps.append({
            "emb": emb_pad, "idx": idx_maps[c], "wx": wx_np, "wh": wh_np,
            "vt": vt_np, "bc": bc_np,
        })
    res = run_bass_kernel_spmd(nc, in_maps, list(range(NCORES)), trace=trace)

    out = np.zeros((B, 2), np.float32)
    for c in range(NCORES):
        logitsT = res.results[c]["out"]  # [2, Q]
        order = orders[c]
        for pos, gi in enumerate(order):
            if gi >= 0:
                out[gi] = logitsT[:, pos]
    return out, res


def kernel(**inputs):
    out, _ = _prep_and_run(inputs, trace=False)
    return out


# revision 25
# speedup vs baseline: 1.1270x; 1.0685x over previous
"""LSTM sequence classifier on 8 Trainium2 NeuronCores.

Data-parallel over batch: each core gets ~1/8 of the 4096 sequences.
Per core: dma_gather (transpose mode, 4 SWDGE queues) pulls token
embeddings from the bf16 table in HBM into feature-major SBUF layout;
a fully unrolled 22-step LSTM runs as bf16 matmuls (fp32 PSUM
accumulate). Biases ride in the matmul via a constant-1 embedding
column, so each gate drains from a 3-bank PSUM tile with a single
ACT instruction (sigmoid/tanh). Cell math runs on DVE in bf16 (2x
mode). Batches are sorted by length (descending) and dealt so all
cores share an identical length multiset; per-step work shrinks to
the still-active prefix.
"""
import sys

sys.path.insert(0, "/opt/trn_rl_repo")

import numpy as np
import ml_dtypes

import concourse.bass as bass
import concourse.tile as tile
from concourse import bacc, mybir
from concourse.bass_utils import run_bass_kernel_spmd

V, E, H, T, B = 30000, 300, 300, 22, 4096
NCORES = 8
EP = 384          # padded embedding row (elements); 768 B in bf16
GP = 384          # padded rows per gate (3 K-tiles of 128)
MW = 4 * GP       # 1536 padded gate rows total
NMT = MW // 128   # 12 M-tiles
KT = 3            # K-tiles per operand (301 -> 128,128,45 incl bias row)
F32 = mybir.dt.float32
BF16 = mybir.dt.bfloat16
FP8 = mybir.dt.float8e4
I16 = mybir.dt.int16
AF = mybir.ActivationFunctionType
WSCALE = 64.0     # weights carried x64 so h/Wh survive fp8; undone at drain

_patched = False


def _patch_tile_drain():
    """walrus CTRL (Drain) supports fewer sem waits than Tile attaches at
    the kernel tail; spread them across single-wait SP NOPs instead."""
    global _patched
    if _patched:
        return
    _patched = True
    import concourse.tile as tile_mod
    from concourse.vector_clock import ScopedClock

    def _drain_and_barrier(self, tick_clock, wait_clock):
        nc = self.nc
        probe = nc.sync.nop(nofuse=True)
        wait_clock.add_sem_waits(
            probe.ins, ScopedClock({None: tick_clock.global_clock}))
        si = probe.ins.sync_info
        waits = list(si.on_wait) if si is not None else []
        upds = list(si.on_update) if si is not None else []
        probe.ins.sync_info = mybir.SyncInfo(on_wait=waits[:1], on_update=upds)
        for w in waits[1:]:
            n2 = nc.sync.nop(nofuse=True)
            n2.ins.sync_info = mybir.SyncInfo(on_wait=[w], on_update=[])
        nc.sync.drain()
        nc.all_engine_barrier()
        popped = nc._tile_sem_poison_stack.pop()
        assert popped is self._sem_poison
        nc.clear_and_free_semaphores(list(self.sems.allocated().values()))
        nc.all_engine_barrier()

    tile_mod.TileContext._drain_and_barrier = _drain_and_barrier


def _schedule(cap_len):
    """Deal batches to cores so every core has the same length multiset.

    Returns orders ([NCORES][Q] of global index or -1 for dummy) and the
    per-step active counts n_t (identical across cores).
    """
    q = np.zeros(T + 1, np.int64)  # q[l] = per-core count of length l
    orders = [[] for _ in range(NCORES)]
    for l in range(T, 0, -1):
        idxs = np.nonzero(cap_len == l)[0]
        k = len(idxs)
        ql = -(-k // NCORES)  # ceil
        q[l] = ql
        for c in range(NCORES):
            part = idxs[c::NCORES]
            orders[c].extend(int(x) for x in part)
            orders[c].extend([-1] * (ql - len(part)))
    n_t = [int(q[t + 1:].sum()) for t in range(T)]  # active at step t
    return orders, n_t


DEBUG_DUMP = False
DRAIN_SPLIT = False


def _build_program(n_t, Q, NTOKP, chunks, offs):
    nc = bacc.Bacc("TRN2", target_bir_lowering=False, debug=False)
    emb_d = nc.dram_tensor("emb", [V, EP], BF16, kind="ExternalInput")
    idx_d = nc.dram_tensor("idx", [128, NTOKP // 16], I16, kind="ExternalInput")
    wx_d = nc.dram_tensor("wx", [KT, 128, MW], BF16, kind="ExternalInput")
    wh_d = nc.dram_tensor("wh", [KT, 128, MW], BF16, kind="ExternalInput")
    wh8_d = nc.dram_tensor("wh8", [2, 128, MW], FP8, kind="ExternalInput")
    vt_d = nc.dram_tensor("vt", [KT, 128, 2], BF16, kind="ExternalInput")
    bc_d = nc.dram_tensor("bc", [2, 1], F32, kind="ExternalInput")
    out_d = nc.dram_tensor("out", [2, Q], F32, kind="ExternalOutput")
    if DEBUG_DUMP:
        QR0 = -(-Q // 8) * 8
        dbg_g = nc.dram_tensor("dbg_g", [4, 128, KT, QR0], F32,
                               kind="ExternalOutput")
        dbg_c = nc.dram_tensor("dbg_c", [2, 128, KT, QR0], F32,
                               kind="ExternalOutput")

    QR = -(-Q // 8) * 8
    SP = Q > 512           # spill columns beyond 512
    SPW = max(8, QR - 512) if SP else 0
    # gate order in the M layout: i, f, g, o (matches reference split)
    # issue order per step: i, g, f, o
    GFUNC = {0: AF.Sigmoid, 1: AF.Sigmoid, 2: AF.Tanh, 3: AF.Sigmoid}
    ISSUE = [0, 2, 1, 3]

    with tile.TileContext(nc) as tc:
        with (
            tc.tile_pool(name="const", bufs=1) as cpool,
            tc.tile_pool(name="xt", bufs=1) as xpool,
            tc.tile_pool(name="state", bufs=1) as spool,
            tc.tile_pool(name="ps", bufs=2, space="PSUM") as pspool,
            tc.tile_pool(name="spill", bufs=1, space="PSUM") as sppool,
            tc.tile_pool(name="psh", bufs=1, space="PSUM") as hpool,
        ):
            # ACT table warm-up: load the sigmoid/tanh table set ASAP so it
            # doesn't serialize behind the first gather.
            warm = spool.tile([2, 8], F32, tag="warm")
            nc.vector.memset(warm[:], 0.0)
            nc.scalar.activation(warm[:], warm[:], AF.Sigmoid)

            wx_sb = cpool.tile([128, KT, MW], BF16, tag="wx")
            wh_sb = cpool.tile([128, KT, MW], BF16, tag="wh")
            wh8_sb = cpool.tile([128, 2, MW], FP8, tag="wh8")
            for k in range(KT):
                nc.sync.dma_start(out=wx_sb[:, k, :], in_=wx_d[k])
                nc.sync.dma_start(out=wh_sb[:, k, :], in_=wh_d[k])
            for q in range(2):
                nc.sync.dma_start(out=wh8_sb[:, q, :], in_=wh8_d[q])
            vt_sb = cpool.tile([128, KT, 2], BF16, tag="vt")
            for k in range(KT):
                nc.sync.dma_start(out=vt_sb[:, k, :], in_=vt_d[k])
            bc_sb = cpool.tile([2, 1], F32, tag="bc")
            nc.sync.dma_start(out=bc_sb[:], in_=bc_d[:])
            idx_sb = cpool.tile([128, NTOKP // 16], I16, tag="idx")
            nc.sync.dma_start(out=idx_sb[:], in_=idx_d[:])

            # gather chunks (feature-major bf16: xt[q, c, i] = emb[tok_i, 128c+q])
            # spread across the 4 SWDGE queues so they land in parallel
            xts = []
            for ci, (s0, s1) in enumerate(chunks):
                xt = xpool.tile([128, KT, s1 - s0], BF16, tag=f"xt{ci}")
                nc.gpsimd.dma_gather(
                    out_ap=xt[:], in_ap=emb_d[:],
                    idxs_ap=idx_sb[:, s0 // 16:s1 // 16],
                    num_idxs=s1 - s0, num_idxs_reg=s1 - s0,
                    elem_size=EP, transpose=True, single_packet=False)
                xts.append(xt)

            hT = spool.tile([128, KT, QR], BF16, tag="hT")
            h8 = spool.tile([128, 2, QR], FP8, tag="h8")
            cT = spool.tile([128, KT, QR], BF16, tag="cT")
            tanh_c = spool.tile([128, KT, QR], BF16, tag="tanh_c")
            tmp = spool.tile([128, KT, QR], BF16, tag="tmp")
            lastT = spool.tile([128, KT, QR], BF16, tag="lastT")
            gbufs = []
            for nm in ["ib", "fb", "gb", "ob"]:
                gt = spool.tile([128, KT, QR], BF16, tag=nm, name=nm)
                gbufs.append(gt)

            def x_segments(off, lo, hi):
                """split cols [lo,hi) at gather-chunk crossings and col 512
                (main PSUM tile vs spill tile boundary). Chunks overlap in
                token space; prefer the chunk that covers the most columns."""
                segs = []
                col = lo
                while col < hi:
                    p = off + col
                    ci = max((i for i, (c0, c1) in enumerate(chunks)
                              if c0 <= p < c1),
                             key=lambda i: chunks[i][1])
                    end = min(hi, chunks[ci][1] - off)
                    if col < 512:
                        end = min(end, 512)
                    segs.append((col, end, ci, p - chunks[ci][0]))
                    col = end
                return segs

            def gate_mms(t, g, off, n, ps, ps_sp):
                """Issue all matmuls for gate g of step t.

                start=True clears has_written for the WHOLE bank, so a
                bank's (x+h) group for one column segment must fully stop
                before the next segment's group starts in that bank. Within
                one segment the x-phase is front-loaded across the 3 banks
                (independent), then the h-phase closes each bank's group.
                """
                has_h = t > 0
                use_dr = has_h and n >= 128  # fp8 DoubleRow pair for h k0/k1
                for (lo, hi, ci, a) in x_segments(off, 0, n):
                    w = hi - lo
                    if lo >= 512:
                        pst, jlo = ps_sp, 512
                    else:
                        pst, jlo = ps, 0
                    for j in range(3):
                        m = g * 3 + j
                        for k in range(KT):
                            nc.tensor.matmul(
                                pst[:, j, lo - jlo:hi - jlo],
                                wx_sb[:, k, m * 128:(m + 1) * 128],
                                xts[ci][:, k, a:a + w],
                                start=(k == 0),
                                stop=(not has_h and k == KT - 1))
                    if has_h and use_dr:
                        for j in range(3):
                            m = g * 3 + j
                            nc.tensor.matmul(
                                pst[:, j, lo - jlo:hi - jlo],
                                wh8_sb[:, :, m * 128:(m + 1) * 128],
                                h8[:, :, lo:hi],
                                start=False, stop=False,
                                perf_mode=mybir.MatmulPerfMode.DoubleRow)
                            nc.tensor.matmul(
                                pst[:, j, lo - jlo:hi - jlo],
                                wh_sb[:, 2, m * 128:(m + 1) * 128],
                                hT[:, 2, lo:hi],
                                start=False, stop=True)
                    elif has_h:
                        for j in range(3):
                            m = g * 3 + j
                            for k in range(KT):
                                nc.tensor.matmul(
                                    pst[:, j, lo - jlo:hi - jlo],
                                    wh_sb[:, k, m * 128:(m + 1) * 128],
                                    hT[:, k, lo:hi],
                                    start=False, stop=(k == KT - 1))

            def gate_drain(g, n, ps, ps_sp, split=False):
                nm = min(n, 512)
                if split or DRAIN_SPLIT:
                    # per-bank drains pipeline with per-bank h-writes
                    for j in range(3):
                        nc.scalar.activation(gbufs[g][:, j, 0:nm],
                                             ps[:, j, 0:nm], GFUNC[g],
                                             scale=1.0 / WSCALE)
                else:
                    nc.scalar.activation(gbufs[g][:, :, 0:nm], ps[:, :, 0:nm],
                                         GFUNC[g], scale=1.0 / WSCALE)
                if n > 512:
                    nc.scalar.activation(gbufs[g][:, :, 512:n],
                                         ps_sp[:, :, 0:n - 512], GFUNC[g],
                                         scale=1.0 / WSCALE)

            for t in range(T):
                n = n_t[t]
                if n == 0:
                    continue
                off = offs[t]
                ib, fb, gb, ob = gbufs
                pss = {}
                spills = {}
                for gi, g in enumerate(ISSUE):
                    ps = pspool.tile([128, 3, 512], F32, tag="ps", name="ps")
                    ps_sp = None
                    if n > 512:
                        ps_sp = sppool.tile([128, 3, SPW], F32, tag="sp",
                                            name="ps_sp")
                    pss[g] = ps
                    spills[g] = ps_sp
                    gate_mms(t, g, off, n, ps, ps_sp)
                    gate_drain(g, n, ps, ps_sp, split=(g == 3 and t < T - 1))
                    # interleave DVE cell math as inputs become ready
                    if gi == 1:  # i and g drained
                        if t == 0:
                            nc.vector.tensor_mul(cT[:, :, :n], ib[:, :, :n],
                                                 gb[:, :, :n])
                        else:
                            nc.vector.tensor_mul(tmp[:, :, :n], ib[:, :, :n],
                                                 gb[:, :, :n])
                    if gi == 2:  # f drained
                        if t > 0:
                            nc.vector.tensor_mul(cT[:, :, :n], fb[:, :, :n],
                                                 cT[:, :, :n])
                            nc.vector.tensor_add(cT[:, :, :n], cT[:, :, :n],
                                                 tmp[:, :, :n])
                        nc.scalar.activation(tanh_c[:, :, :n], cT[:, :, :n],
                                             AF.Tanh)
                # o drained: produce h and capture finished sequences
                cap_lo = n_t[t + 1] if t < T - 1 else 0
                if t < T - 1 and cap_lo > 0:
                    next_dr = n_t[t + 1] >= 128
                    for j in range(3):
                        dst = (h8 if (next_dr and j < 2) else hT)
                        nc.vector.tensor_mul(dst[:, j, :cap_lo],
                                             ob[:, j, :cap_lo],
                                             tanh_c[:, j, :cap_lo])
                if cap_lo < n:
                    nc.vector.tensor_mul(lastT[:, :, cap_lo:n],
                                         ob[:, :, cap_lo:n],
                                         tanh_c[:, :, cap_lo:n])
                if DEBUG_DUMP and t == 0:
                    dbg_sb = spool.tile([128, KT, QR], F32, tag="dbg_sb")
                    for gi in range(4):
                        nc.vector.tensor_copy(dbg_sb[:], gbufs[gi][:])
                        nc.sync.dma_start(out=dbg_g[gi], in_=dbg_sb[:])
                    nc.vector.tensor_copy(dbg_sb[:], cT[:])
                    nc.sync.dma_start(out=dbg_c[0], in_=dbg_sb[:])
                    nc.vector.tensor_copy(dbg_sb[:], hT[:])
                    nc.sync.dma_start(out=dbg_c[1], in_=dbg_sb[:])

            # head: logits^T = W @ last^T + b_cls (W precomputed on host)
            out_sb = spool.tile([2, QR], F32, tag="out_sb")
            col = 0
            while col < Q:
                w = min(512, Q - col)
                ph = hpool.tile([2, 512], F32, tag="ph")
                for k in range(KT):
                    nc.tensor.matmul(ph[:, :w], vt_sb[:, k, :],
                                     lastT[:, k, col:col + w],
                                     start=(k == 0), stop=(k == KT - 1))
                nc.scalar.activation(out_sb[:, col:col + w], ph[:, :w],
                                     AF.Identity, bias=bc_sb[:, 0:1],
                                     scale=1.0)
                col += w
            nc.sync.dma_start(out=out_d[:], in_=out_sb[:, :Q])

    nc.compile()
    return nc


def _prep_and_run(inputs, trace=False):
    _patch_tile_drain()
    cap = np.asarray(inputs["cap"]).astype(np.int64)
    cap_len = np.asarray(inputs["cap_len"]).astype(np.int64)
    embed = np.asarray(inputs["embed"], np.float32)
    W_ih = np.asarray(inputs["W_ih"], np.float32)
    W_hh = np.asarray(inputs["W_hh"], np.float32)
    b_ih = np.asarray(inputs["b_ih"], np.float32)
    b_hh = np.asarray(inputs["b_hh"], np.float32)
    v_wn = np.asarray(inputs["v_wn"], np.float32)
    g_wn = np.asarray(inputs["g_wn"], np.float32)
    b_cls = np.asarray(inputs["b_cls"], np.float32)

    orders, n_t = _schedule(cap_len)
    Q = n_t[0]
    offs = np.concatenate([[0], np.cumsum(n_t)]).astype(np.int64)
    NTOK = int(offs[-1])
    NTOKP = -(-NTOK // 128) * 128

    # per-core token streams, packed for dma_gather (idx i -> [i%16, i//16])
    idx_maps = []
    for c in range(NCORES):
        order = np.asarray(orders[c], np.int64)
        toks = np.zeros(NTOKP, np.int16)
        for t in range(T):
            n = n_t[t]
            sel = order[:n]
            tk = np.where(sel >= 0, cap[np.clip(sel, 0, None), t], 0)
            toks[offs[t]:offs[t] + n] = tk.astype(np.int16)
        packed = np.tile(toks.reshape(NTOKP // 16, 16).T, (8, 1)).copy()
        idx_maps.append(packed)

    # step-aligned overlapping gather chunks: each chunk is a 128-aligned
    # superset of a group of steps' token ranges, so steps t>=1 read their
    # x in a single segment (no extra matmul splits at chunk crossings).
    def fl128(v):
        return (v // 128) * 128

    def cl128(v):
        return -(-v // 128) * 128

    chunks = []
    e0 = min(cl128(int(offs[1])), NTOKP)
    if e0 > 256:
        chunks += [(0, 256), (256, e0)]
    else:
        chunks += [(0, max(e0, 128))]
    for grp in ([1], [2, 3], [4, 5, 6], [7, 8, 9, 10], list(range(11, T))):
        ta, tb = grp[0], grp[-1]
        lo, hi = int(offs[ta]), int(offs[tb + 1])
        if hi <= lo:
            continue
        chunks.append((fl128(lo), min(cl128(hi), NTOKP)))

    # embedding table: bf16, padded to EP with a constant-1 bias column at
    # index 300 (so W rows at k-row 300 add the LSTM bias inside the matmul)
    emb_pad = np.zeros((V, EP), ml_dtypes.bfloat16)
    emb_pad[:, :E] = embed.astype(ml_dtypes.bfloat16)
    emb_pad[:, E] = np.float32(1.0)

    def pack_w(Wmat, kdim, bias=None):
        Wp = np.zeros((MW, EP), np.float32)
        for g in range(4):
            Wp[GP * g:GP * g + H, :kdim] = Wmat[H * g:H * g + H, :]
            if bias is not None:
                Wp[GP * g:GP * g + H, E] = bias[H * g:H * g + H]
        return np.ascontiguousarray(
            Wp.T.reshape(KT, 128, MW)).astype(ml_dtypes.bfloat16)

    wx_np = pack_w(W_ih * WSCALE, E, bias=(b_ih + b_hh) * WSCALE)
    wh_np = pack_w(W_hh * WSCALE, H)
    # fp8 DoubleRow lhsT for h k-tiles 0,1: [plane, 128, MW]
    Whp = np.zeros((MW, 256), np.float32)
    for g in range(4):
        Whp[GP * g:GP * g + H, :] = W_hh[H * g:H * g + H, :256] * WSCALE
    wh8_np = np.ascontiguousarray(
        Whp.T.reshape(2, 128, MW)).astype(ml_dtypes.float8_e4m3fn)

    # weight-normed head, computed on host: W = g * v / ||v||
    Wv = (g_wn[:, None] * v_wn / np.linalg.norm(v_wn, axis=1, keepdims=True))
    v_pad = np.zeros((2, EP), np.float32)
    v_pad[:, :H] = Wv
    vt_np = np.ascontiguousarray(
        v_pad.T.reshape(KT, 128, 2)).astype(ml_dtypes.bfloat16)
    bc_np = np.ascontiguousarray(b_cls.reshape(2, 1)).astype(np.float32)

    nc = _build_program(n_t, Q, NTOKP, chunks, offs)

    in_maps = []
    for c in range(NCORES):
        in_maps.append({
            "emb": emb_pad, "idx": idx_maps[c], "wx": wx_np, "wh": wh_np,
            "wh8": wh8_np, "vt": vt_np, "bc": bc_np,
        })
    res = run_bass_kernel_spmd(nc, in_maps, list(range(NCORES)), trace=trace)

    out = np.zeros((B, 2), np.float32)
    for c in range(NCORES):
        logitsT = res.results[c]["out"]  # [2, Q]
        order = orders[c]
        for pos, gi in enumerate(order):
            if gi >= 0:
                out[gi] = logitsT[:, pos]
    return out, res


def kernel(**inputs):
    out, _ = _prep_and_run(inputs, trace=False)
    return out
